# revision 26
# baseline (speedup 1.0000x reference)
"""Multi-head causal attention (B=2, L=2048, D=1024, H=16) on 8 trn2 cores.

Sharding: data-parallel over batch (2) x tensor-parallel over heads (4 groups
of 4 heads).  Core c handles batch c//4, heads 4*(c%4) .. 4*(c%4)+3.
Wq/Wk/Wv are column-sharded, Wo row-sharded; the TP all-reduce after Wo is
done host-side (sum of the 4 partial outputs per batch), as is the bo add.

Per-core kernel (matmul operands bf16, all accumulation fp32 in PSUM):
  - host supplies X^T (D on partitions) so projections need no on-chip
    transpose: Q^T/K^T = matmul(lhsT=W, rhs=X^T), V = matmul(lhsT=X^T, rhs=W)
  - S^T[k, q] tiles = matmul(lhsT=K^T tile, rhs=Q^T strip); softmax is
    computed WITHOUT max-subtraction (scores are ~N(0,0.6), bounded by ~4,
    so exp is safe) which keeps all reductions off the partition axis
  - P^T = exp(S^T/8) directly in the layout PV needs; a ones-row appended to
    V (lhsT [128, 65]) makes the PV matmul also emit softmax denominators
  - normalization: reciprocal of the denom row, partition-broadcast via a
    K=1 matmul (ones x recip), multiplied into O^T before the Wo matmul
  - causal mask: S^T/exp/PV restricted to q >= k-tile start; the single
    diagonal 128x128 block is masked by multiplying P^T with an upper-
    triangular 0/1 tile (supplied as input)
  - two heads are processed per S^T psum tile so each exp instruction covers
    [128, 2, 512-lo] (ACT per-instruction overhead is ~370ns)
"""

import numpy as np

B, L, D, H = 2, 2048, 1024, 16
DK = D // H          # 64
NCORES = 8
TP = 4               # head-group shards per batch
HG = H // TP         # 4 heads per core
DH = HG * DK         # 256 per-core head dims
STRIP = 512          # attention q-strip width
NSTRIP = L // STRIP  # 4
SUB = 512            # projection substrip width
NSUB = L // SUB      # 4
KT = 128             # key tile
NKT = L // KT        # 16

_CACHE = {}


def _build(causal: bool):
    import concourse.mybir as mybir
    import concourse.tile as tile
    from concourse import bacc

    f32 = mybir.dt.float32
    f32r = mybir.dt.float32r
    bf16 = mybir.dt.bfloat16
    EXP = mybir.ActivationFunctionType.Exp

    nc = bacc.Bacc("TRN2", target_bir_lowering=False)

    qT = nc.dram_tensor("qT", [D, L], bf16, kind="ExternalInput")
    kT = nc.dram_tensor("kT", [D, L], bf16, kind="ExternalInput")
    vT = nc.dram_tensor("vT", [D, L], bf16, kind="ExternalInput")
    wq = nc.dram_tensor("wq", [D, DH], bf16, kind="ExternalInput")
    wk = nc.dram_tensor("wk", [D, DH], bf16, kind="ExternalInput")
    wv = nc.dram_tensor("wv", [D, DH], bf16, kind="ExternalInput")
    wo = nc.dram_tensor("wo", [DH, D], bf16, kind="ExternalInput")
    bq = nc.dram_tensor("bq", [DH], f32, kind="ExternalInput")
    bk = nc.dram_tensor("bk", [DH], f32, kind="ExternalInput")
    bv = nc.dram_tensor("bv", [DH], bf16, kind="ExternalInput")
    tri = nc.dram_tensor("tri", [KT, KT], bf16, kind="ExternalInput")
    maskT = None
    if not causal:
        maskT = nc.dram_tensor("maskT", [L, L], bf16, kind="ExternalInput")
    f16 = mybir.dt.float16
    out = nc.dram_tensor("out", [L, D], f16, kind="ExternalOutput")

    with tile.TileContext(nc) as tc:
        with (
            tc.tile_pool(name="consts", bufs=1) as consts,
            tc.tile_pool(name="resident", bufs=1) as resident,
            tc.tile_pool(name="xin", bufs=2) as xin,
            tc.tile_pool(name="ptile", bufs=6) as ptile,
            tc.tile_pool(name="small", bufs=2) as small,
            tc.tile_pool(name="stage", bufs=2) as stage,
            tc.tile_pool(name="mtile", bufs=4) as mtile,
            tc.tile_pool(name="bank", bufs=4, space="PSUM") as bank,
            tc.tile_pool(name="sps", bufs=2, space="PSUM") as sps,
        ):
            # ---- constants / weights ----
            wq_t = consts.tile([128, 8, DH], bf16, tag="wq")
            wk_t = consts.tile([128, 8, DH], bf16, tag="wk")
            wv_t = consts.tile([128, 8, DH], bf16, tag="wv")
            wo_t = consts.tile([128, 2, D], bf16, tag="wo")
            xa = xin.tile([128, 8, SUB], bf16, tag="xq", name="xq_pre0")
            xb = xin.tile([128, 8, SUB], bf16, tag="xk", name="xk_pre0")
            xc = xin.tile([128, 8, SUB], bf16, tag="xv", name="xv_pre0")
            x_first = [(xa, xb, xc)]
            ssl = slice(0, SUB)
            nc.sync.dma_start(out=wq_t, in_=wq.rearrange("(c p) m -> p c m", p=128))
            nc.sync.dma_start(out=x_first[0][0], in_=qT[:, ssl].rearrange("(c p) n -> p c n", p=128))
            nc.sync.dma_start(out=wk_t, in_=wk.rearrange("(c p) m -> p c m", p=128))
            nc.sync.dma_start(out=x_first[0][1], in_=kT[:, ssl].rearrange("(c p) n -> p c n", p=128))
            nc.sync.dma_start(out=wv_t, in_=wv.rearrange("(c p) m -> p c m", p=128))
            nc.sync.dma_start(out=x_first[0][2], in_=vT[:, ssl].rearrange("(c p) n -> p c n", p=128))
            nc.sync.dma_start(out=wo_t, in_=wo.rearrange("(c p) n -> p c n", p=128))
            # per-partition bias columns for the q/k PSUM->SBUF copy
            bqP = consts.tile([128, 2], f32, tag="bqP")
            bkP = consts.tile([128, 2], f32, tag="bkP")
            nc.sync.dma_start(out=bqP, in_=bq.rearrange("(m p) -> p m", p=128))
            nc.sync.dma_start(out=bkP, in_=bk.rearrange("(m p) -> p m", p=128))
            bv_t = consts.tile([1, DH], bf16, tag="bv")
            nc.sync.dma_start(out=bv_t, in_=bv[:].unsqueeze(0))
            tri_t = consts.tile([KT, KT], bf16, tag="tri")
            nc.sync.dma_start(out=tri_t, in_=tri[:])
            ones_f = consts.tile([1, 128], f32, tag="ones")
            nc.vector.memset(ones_f, 1.0)
            ones_r = ones_f[:, :].bitcast(f32r)
            ones_b = consts.tile([1, 128], bf16, tag="onesb")
            nc.vector.memset(ones_b, 1.0)

            # ---- resident activations (one tile per strip to keep
            # scheduler dependencies fine-grained) ----
            # q_s/k_s: [partition = (h%2)*64 + dk, pair = h//2, q-in-strip]
            q_s, k_s, v_s, o_s = [], [], [], []
            for s in range(NSTRIP):
                q_tile = resident.tile([128, 2, STRIP], bf16, tag=f"q{s}", name=f"q{s}")
                k_tile = resident.tile([128, 2, STRIP], bf16, tag=f"k{s}", name=f"k{s}")
                # v: [partition = k within tile, ktile-in-strip, head, dk+1]
                v_tile = resident.tile([128, 4, HG, DK + 1], bf16, tag=f"v{s}", name=f"v{s}")
                nc.vector.memset(v_tile[:, :, :, DK : DK + 1], 1.0)
                o_tile = resident.tile([128, 2, STRIP], bf16, tag=f"o{s}", name=f"o{s}")
                q_s.append(q_tile); k_s.append(k_tile); v_s.append(v_tile); o_s.append(o_tile)

            def proj_q(t):
                ctx_ = nc.named_scope(f"projq{t}"); ctx_.__enter__()
                if t == 0:
                    x_q = x_first[0][0]
                else:
                    x_q = xin.tile([128, 8, SUB], bf16, tag="xq")
                    sl = slice(t * SUB, (t + 1) * SUB)
                    nc.sync.dma_start(out=x_q, in_=qT[:, sl].rearrange("(c p) n -> p c n", p=128))
                for m in range(2):
                    ps = bank.tile([128, SUB], f32, tag="bank")
                    msl = slice(m * 128, (m + 1) * 128)
                    for c in range(8):
                        nc.tensor.matmul(ps, lhsT=wq_t[:, c, msl], rhs=x_q[:, c, :],
                                         start=(c == 0), stop=(c == 7))
                    nc.vector.tensor_scalar_add(q_s[t][:, m, :], ps, bqP[:, m:m + 1])
                ctx_.__exit__(None, None, None)

            def proj_kv(t):
                ctx_ = nc.named_scope(f"projkv{t}"); ctx_.__enter__()
                if t == 0:
                    x_k, x_v = x_first[0][1], x_first[0][2]
                else:
                    x_k = xin.tile([128, 8, SUB], bf16, tag="xk")
                    x_v = xin.tile([128, 8, SUB], bf16, tag="xv")
                    sl = slice(t * SUB, (t + 1) * SUB)
                    nc.sync.dma_start(out=x_k, in_=kT[:, sl].rearrange("(c p) n -> p c n", p=128))
                    nc.sync.dma_start(out=x_v, in_=vT[:, sl].rearrange("(c p) n -> p c n", p=128))
                for m in range(2):
                    ps = bank.tile([128, SUB], f32, tag="bank")
                    msl = slice(m * 128, (m + 1) * 128)
                    for c in range(8):
                        nc.tensor.matmul(ps, lhsT=wk_t[:, c, msl], rhs=x_k[:, c, :],
                                         start=(c == 0), stop=(c == 7))
                    nc.vector.tensor_scalar_add(k_s[t][:, m, :], ps, bkP[:, m:m + 1])
                for j in range(4):
                    ps = bank.tile([128, DH], f32, tag="bank")
                    nc.tensor.matmul(ps, lhsT=ones_b, rhs=bv_t,
                                     start=True, stop=False)
                    for c in range(8):
                        nc.tensor.matmul(ps, lhsT=x_v[:, c, j * 128:(j + 1) * 128],
                                         rhs=wv_t[:, c, :], start=False, stop=(c == 7))
                    nc.vector.tensor_copy(
                        v_s[t][:, j, :, 0:DK],
                        ps.rearrange("p (h d) -> p h d", h=HG),
                    )
                ctx_.__exit__(None, None, None)

            def attention_hp(s, hp):
                ctx_ = nc.named_scope(f"attn{s}h{hp}"); ctx_.__enter__()
                q0 = s * STRIP
                a_max = 4 * s + 3 if causal else NKT - 1
                if True:
                    o_ps0 = bank.tile([65, STRIP], f32, tag="bank")
                    o_ps1 = bank.tile([65, STRIP], f32, tag="bank")
                    o_ps = [o_ps0, o_ps1]
                    for a in range(a_max + 1):
                        lo = max((a - 4 * s) * KT, 0) if causal else 0
                        sp = sps.tile([128, 2, STRIP], f32, tag="sps")
                        for i in range(2):
                            pr = slice(i * 64, (i + 1) * 64)
                            nc.tensor.matmul(
                                sp[:, i, lo:STRIP],
                                lhsT=k_s[a // 4][pr, hp, (a % 4) * KT:(a % 4 + 1) * KT],
                                rhs=q_s[s][pr, hp, lo:STRIP],
                                start=True, stop=True,
                            )
                        pt = ptile.tile([128, 2, STRIP], bf16, tag="pt")
                        nc.scalar.activation(out=pt[:, :, lo:STRIP],
                                             in_=sp[:, :, lo:STRIP],
                                             func=EXP, scale=0.125)
                        if causal and 0 <= a - 4 * s <= 3:
                            d0 = (a - 4 * s) * KT
                            for i in range(2):
                                nc.vector.tensor_mul(pt[:, i, d0:d0 + KT],
                                                     pt[:, i, d0:d0 + KT], tri_t)
                        if not causal:
                            mt = mtile.tile([128, STRIP], bf16, tag="mt")
                            nc.sync.dma_start(
                                out=mt, in_=maskT[a * KT:(a + 1) * KT, q0:q0 + STRIP])
                            for i in range(2):
                                nc.vector.tensor_mul(pt[:, i, :], pt[:, i, :], mt)
                        for i in range(2):
                            nc.tensor.matmul(o_ps[i][:, lo:STRIP],
                                             lhsT=v_s[a // 4][:, a % 4, 2 * hp + i, :],
                                             rhs=pt[:, i, lo:STRIP],
                                             start=(a == 0), stop=(a == a_max))
                    # normalize: recip of denom row, broadcast over 64 partitions
                    for i in range(2):
                        h = 2 * hp + i
                        r_t = small.tile([1, STRIP], f32r, tag="recip")
                        with nc.allow_low_precision(reason="float32r is fp32 bits"):
                            if causal:
                                nc.vector.reciprocal(r_t, o_ps[i][64:65, :])
                            else:
                                dn = small.tile([1, STRIP], f32r, tag="denom")
                                nc.vector.tensor_scalar_max(dn, o_ps[i][64:65, :], 1e-30)
                                nc.vector.reciprocal(r_t, dn)
                        bc_ps = bank.tile([64, STRIP], f32, tag="bank")
                        nc.tensor.matmul(bc_ps, lhsT=ones_r[0:1, 0:64], rhs=r_t,
                                         start=True, stop=True)
                        bc_t = small.tile([64, STRIP], f32, tag="bc")
                        nc.vector.tensor_copy(bc_t, bc_ps)
                        nc.vector.tensor_mul(
                            o_s[s][i * 64:(i + 1) * 64, hp, :],
                            o_ps[i][0:64, :], bc_t)

                ctx_.__exit__(None, None, None)

            def wo_strip(s):
                ctx_ = nc.named_scope(f"wo{s}"); ctx_.__enter__()
                st = stage.tile([128, 4, D], f16, tag="st")
                for t4 in range(4):
                    csl = slice(t4 * 128, (t4 + 1) * 128)
                    for n in range(2):
                        wps = bank.tile([128, 512], f32, tag="bank")
                        nsl = slice(n * 512, (n + 1) * 512)
                        for c in range(2):
                            nc.tensor.matmul(wps, lhsT=o_s[s][:, c, csl],
                                             rhs=wo_t[:, c, nsl],
                                             start=(c == 0), stop=(c == 1))
                        if (t4 + n) % 2 == 0:
                            nc.scalar.copy(out=st[:, t4, nsl], in_=wps)
                        else:
                            nc.vector.tensor_copy(st[:, t4, nsl], wps)
                nc.sync.dma_start(
                    out=out[s * STRIP:(s + 1) * STRIP, :].rearrange(
                        "(t p) n -> p t n", p=128),
                    in_=st,
                )
                ctx_.__exit__(None, None, None)

            if causal:
                proj_q(0)
                proj_kv(0)
                proj_q(1)
                proj_kv(1)
                with tc.high_priority():
                    attention_hp(0, 0)
                    attention_hp(0, 1)
                    wo_strip(0)
                proj_q(2)
                proj_kv(2)
                with tc.high_priority():
                    attention_hp(1, 0)
                    attention_hp(1, 1)
                    wo_strip(1)
                proj_q(3)
                proj_kv(3)
                with tc.high_priority():
                    attention_hp(2, 0)
                    attention_hp(2, 1)
                    wo_strip(2)
                    attention_hp(3, 0)
                    attention_hp(3, 1)
                    wo_strip(3)
            else:
                # non-causal: every strip reads every K/V tile, so all
                # projections must be emitted before any attention
                for t in range(NSUB):
                    proj_q(t)
                    proj_kv(t)
                for s in range(NSTRIP):
                    attention_hp(s, 0)
                    attention_hp(s, 1)
                    wo_strip(s)

    nc.compile()
    return nc


def _get_kernel(causal: bool):
    key = ("attn", causal)
    if key not in _CACHE:
        _CACHE[key] = _build(causal)
    return _CACHE[key]


def kernel(query, key, value, mask, wq, bq, wk, bk, wv, bv, wo, bo):
    import ml_dtypes
    from concourse import bass_utils

    f32 = np.float32
    bf16 = ml_dtypes.bfloat16

    mask_b = np.asarray(mask, dtype=bool)
    causal = bool(
        (mask_b[:, 0] == np.tril(np.ones((L, L), dtype=bool))[None]).all()
    )
    nc = _get_kernel(causal)

    tri_np = np.triu(np.ones((KT, KT), dtype=f32)).astype(bf16)
    qT = [np.ascontiguousarray(np.asarray(query[b], f32).T).astype(bf16) for b in range(B)]
    kT = [np.ascontiguousarray(np.asarray(key[b], f32).T).astype(bf16) for b in range(B)]
    vT = [np.ascontiguousarray(np.asarray(value[b], f32).T).astype(bf16) for b in range(B)]
    if not causal:
        maskT = [
            np.ascontiguousarray(mask_b[b, 0].T).astype(bf16) for b in range(B)
        ]

    wq = np.asarray(wq, f32)
    wk = np.asarray(wk, f32)
    wv = np.asarray(wv, f32)
    wo = np.asarray(wo, f32)
    bq = np.asarray(bq, f32)
    bk = np.asarray(bk, f32)
    bv = np.asarray(bv, f32)

    in_maps = []
    for c in range(NCORES):
        b, g = c // TP, c % TP
        gs = slice(g * DH, (g + 1) * DH)
        m = {
            "qT": qT[b], "kT": kT[b], "vT": vT[b],
            "wq": np.ascontiguousarray(wq[:, gs]).astype(bf16),
            "wk": np.ascontiguousarray(wk[:, gs]).astype(bf16),
            "wv": np.ascontiguousarray(wv[:, gs]).astype(bf16),
            "wo": np.ascontiguousarray(wo[gs, :]).astype(bf16),
            "bq": np.ascontiguousarray(bq[gs]),
            "bk": np.ascontiguousarray(bk[gs]),
            "bv": np.ascontiguousarray(bv[gs]).astype(bf16),
            "tri": tri_np,
        }
        if not causal:
            m["maskT"] = maskT[b]
        in_maps.append(m)

    res = bass_utils.run_bass_kernel_spmd(nc, in_maps, core_ids=list(range(NCORES)))

    out = np.zeros((B, L, D), f32)
    for c in range(NCORES):
        out[c // TP] += res.results[c]["out"].astype(f32)
    out += np.asarray(bo, f32)[None, None, :]
    return out


# revision 34
# speedup vs baseline: 1.0306x; 1.0306x over previous
"""Multi-head causal attention (B=2, L=2048, D=1024, H=16) on 8 trn2 cores.

Sharding: data-parallel over batch (2) x tensor-parallel over heads (4 groups
of 4 heads).  Core c handles batch c//4, heads 4*(c%4) .. 4*(c%4)+3.
Wq/Wk/Wv are column-sharded, Wo row-sharded; the TP all-reduce after Wo is
done host-side (sum of the 4 partial outputs per batch), as is the bo add.

Per-core kernel (matmul operands bf16, all accumulation fp32 in PSUM):
  - host supplies X^T (D on partitions) so projections need no on-chip
    transpose: Q^T/K^T = matmul(lhsT=W, rhs=X^T), V = matmul(lhsT=X^T, rhs=W)
  - S^T[k, q] tiles = matmul(lhsT=K^T tile, rhs=Q^T strip); softmax is
    computed WITHOUT max-subtraction (scores are ~N(0,0.6), bounded by ~4,
    so exp is safe) which keeps all reductions off the partition axis
  - P^T = exp(S^T/8) directly in the layout PV needs; a ones-row appended to
    V (lhsT [128, 65]) makes the PV matmul also emit softmax denominators
  - normalization: reciprocal of the denom row, partition-broadcast via a
    K=1 matmul (ones x recip), multiplied into O^T before the Wo matmul
  - causal mask: S^T/exp/PV restricted to q >= k-tile start; the single
    diagonal 128x128 block is masked by multiplying P^T with an upper-
    triangular 0/1 tile (supplied as input)
  - two heads are processed per S^T psum tile so each exp instruction covers
    [128, 2, 512-lo] (ACT per-instruction overhead is ~370ns)
"""

import numpy as np

B, L, D, H = 2, 2048, 1024, 16
DK = D // H          # 64
NCORES = 8
TP = 4               # head-group shards per batch
HG = H // TP         # 4 heads per core
DH = HG * DK         # 256 per-core head dims
STRIP = 512          # attention q-strip width
NSTRIP = L // STRIP  # 4
SUB = 512            # projection substrip width
NSUB = L // SUB      # 4
KT = 128             # key tile
NKT = L // KT        # 16

_CACHE = {}


def _build(causal: bool):
    import concourse.mybir as mybir
    import concourse.tile as tile
    from concourse import bacc

    f32 = mybir.dt.float32
    f32r = mybir.dt.float32r
    bf16 = mybir.dt.bfloat16
    EXP = mybir.ActivationFunctionType.Exp

    nc = bacc.Bacc("TRN2", target_bir_lowering=False)

    qT = nc.dram_tensor("qT", [D, L], bf16, kind="ExternalInput")
    kT = nc.dram_tensor("kT", [D, L], bf16, kind="ExternalInput")
    vT = nc.dram_tensor("vT", [D, L], bf16, kind="ExternalInput")
    wq = nc.dram_tensor("wq", [D, DH], bf16, kind="ExternalInput")
    wk = nc.dram_tensor("wk", [D, DH], bf16, kind="ExternalInput")
    wv = nc.dram_tensor("wv", [D, DH], bf16, kind="ExternalInput")
    wo = nc.dram_tensor("wo", [DH, D], bf16, kind="ExternalInput")
    bq = nc.dram_tensor("bq", [DH], f32, kind="ExternalInput")
    bk = nc.dram_tensor("bk", [DH], f32, kind="ExternalInput")
    bv = nc.dram_tensor("bv", [DH], bf16, kind="ExternalInput")
    tri = nc.dram_tensor("tri", [KT, KT], bf16, kind="ExternalInput")
    maskT = None
    if not causal:
        maskT = nc.dram_tensor("maskT", [L, L], bf16, kind="ExternalInput")
    f16 = mybir.dt.float16
    out = nc.dram_tensor("out", [L, D], f16, kind="ExternalOutput")

    with tile.TileContext(nc) as tc:
        with (
            tc.tile_pool(name="consts", bufs=1) as consts,
            tc.tile_pool(name="resident", bufs=1) as resident,
            tc.tile_pool(name="xin", bufs=2) as xin,
            tc.tile_pool(name="ptile", bufs=6) as ptile,
            tc.tile_pool(name="small", bufs=2) as small,
            tc.tile_pool(name="stage", bufs=2) as stage,
            tc.tile_pool(name="mtile", bufs=4) as mtile,
            tc.tile_pool(name="bank", bufs=4, space="PSUM") as bank,
            tc.tile_pool(name="sps", bufs=2, space="PSUM") as sps,
        ):
            # ---- constants / weights ----
            wq_t = consts.tile([128, 8, DH], bf16, tag="wq")
            wk_t = consts.tile([128, 8, DH], bf16, tag="wk")
            wv_t = consts.tile([128, 8, DH], bf16, tag="wv")
            wo_t = consts.tile([128, 2, D], bf16, tag="wo")
            xa = xin.tile([128, 8, SUB], bf16, tag="xq", name="xq_pre0")
            xb = xin.tile([128, 8, SUB], bf16, tag="xk", name="xk_pre0")
            xc = xin.tile([128, 8, SUB], bf16, tag="xv", name="xv_pre0")
            x_first = [(xa, xb, xc)]
            ssl = slice(0, SUB)
            nc.sync.dma_start(out=wq_t, in_=wq.rearrange("(c p) m -> p c m", p=128))
            nc.sync.dma_start(out=x_first[0][0], in_=qT[:, ssl].rearrange("(c p) n -> p c n", p=128))
            nc.sync.dma_start(out=wk_t, in_=wk.rearrange("(c p) m -> p c m", p=128))
            nc.sync.dma_start(out=x_first[0][1], in_=kT[:, ssl].rearrange("(c p) n -> p c n", p=128))
            nc.sync.dma_start(out=wv_t, in_=wv.rearrange("(c p) m -> p c m", p=128))
            nc.sync.dma_start(out=x_first[0][2], in_=vT[:, ssl].rearrange("(c p) n -> p c n", p=128))
            nc.sync.dma_start(out=wo_t, in_=wo.rearrange("(c p) n -> p c n", p=128))
            # per-partition bias columns for the q/k PSUM->SBUF copy
            bqP = consts.tile([128, 2], f32, tag="bqP")
            bkP = consts.tile([128, 2], f32, tag="bkP")
            nc.sync.dma_start(out=bqP, in_=bq.rearrange("(m p) -> p m", p=128))
            nc.sync.dma_start(out=bkP, in_=bk.rearrange("(m p) -> p m", p=128))
            bv_t = consts.tile([1, DH], bf16, tag="bv")
            nc.sync.dma_start(out=bv_t, in_=bv[:].unsqueeze(0))
            tri_t = consts.tile([KT, KT], bf16, tag="tri")
            nc.sync.dma_start(out=tri_t, in_=tri[:])
            ones_f = consts.tile([1, 128], f32, tag="ones")
            nc.vector.memset(ones_f, 1.0)
            ones_r = ones_f[:, :].bitcast(f32r)
            ones_b = consts.tile([1, 128], bf16, tag="onesb")
            nc.vector.memset(ones_b, 1.0)

            # ---- resident activations (one tile per strip to keep
            # scheduler dependencies fine-grained) ----
            # q_s/k_s: [partition = (h%2)*64 + dk, pair = h//2, q-in-strip]
            q_s, k_s, v_s, o_s = [], [], [], []
            for s in range(NSTRIP):
                q_tile = resident.tile([128, 2, STRIP], bf16, tag=f"q{s}", name=f"q{s}")
                k_tile = resident.tile([128, 2, STRIP], bf16, tag=f"k{s}", name=f"k{s}")
                # v: [partition = k within tile, ktile-in-strip, head, dk+1]
                v_tile = resident.tile([128, 4, HG, DK + 1], bf16, tag=f"v{s}", name=f"v{s}")
                nc.vector.memset(v_tile[:, :, :, DK : DK + 1], 1.0)
                o_tile = resident.tile([128, 2, STRIP], bf16, tag=f"o{s}", name=f"o{s}")
                q_s.append(q_tile); k_s.append(k_tile); v_s.append(v_tile); o_s.append(o_tile)

            def proj_q(t):
                ctx_ = nc.named_scope(f"projq{t}"); ctx_.__enter__()
                if t == 0:
                    x_q = x_first[0][0]
                else:
                    x_q = xin.tile([128, 8, SUB], bf16, tag="xq")
                    sl = slice(t * SUB, (t + 1) * SUB)
                    nc.sync.dma_start(out=x_q, in_=qT[:, sl].rearrange("(c p) n -> p c n", p=128))
                for m in range(2):
                    ps = bank.tile([128, SUB], f32, tag="bank")
                    msl = slice(m * 128, (m + 1) * 128)
                    for c in range(8):
                        nc.tensor.matmul(ps, lhsT=wq_t[:, c, msl], rhs=x_q[:, c, :],
                                         start=(c == 0), stop=(c == 7))
                    nc.vector.tensor_scalar_add(q_s[t][:, m, :], ps, bqP[:, m:m + 1])
                ctx_.__exit__(None, None, None)

            def proj_k(t):
                ctx_ = nc.named_scope(f"projk{t}"); ctx_.__enter__()
                if t == 0:
                    x_k = x_first[0][1]
                else:
                    x_k = xin.tile([128, 8, SUB], bf16, tag="xk")
                    sl = slice(t * SUB, (t + 1) * SUB)
                    nc.sync.dma_start(out=x_k, in_=kT[:, sl].rearrange("(c p) n -> p c n", p=128))
                for m in range(2):
                    ps = bank.tile([128, SUB], f32, tag="bank")
                    msl = slice(m * 128, (m + 1) * 128)
                    for c in range(8):
                        nc.tensor.matmul(ps, lhsT=wk_t[:, c, msl], rhs=x_k[:, c, :],
                                         start=(c == 0), stop=(c == 7))
                    nc.vector.tensor_scalar_add(k_s[t][:, m, :], ps, bkP[:, m:m + 1])
                ctx_.__exit__(None, None, None)

            def proj_v(t):
                ctx_ = nc.named_scope(f"projv{t}"); ctx_.__enter__()
                if t == 0:
                    x_v = x_first[0][2]
                else:
                    x_v = xin.tile([128, 8, SUB], bf16, tag="xv")
                    sl = slice(t * SUB, (t + 1) * SUB)
                    nc.sync.dma_start(out=x_v, in_=vT[:, sl].rearrange("(c p) n -> p c n", p=128))
                for j in range(4):
                    ps = bank.tile([128, DH], f32, tag="bank")
                    nc.tensor.matmul(ps, lhsT=ones_b, rhs=bv_t,
                                     start=True, stop=False)
                    for c in range(8):
                        nc.tensor.matmul(ps, lhsT=x_v[:, c, j * 128:(j + 1) * 128],
                                         rhs=wv_t[:, c, :], start=False, stop=(c == 7))
                    nc.vector.tensor_copy(
                        v_s[t][:, j, :, 0:DK],
                        ps.rearrange("p (h d) -> p h d", h=HG),
                    )
                ctx_.__exit__(None, None, None)

            def proj_kv(t):
                proj_k(t)
                proj_v(t)

            def attention_hp(s, hp):
                ctx_ = nc.named_scope(f"attn{s}h{hp}"); ctx_.__enter__()
                q0 = s * STRIP
                a_max = 4 * s + 3 if causal else NKT - 1
                if True:
                    o_ps0 = bank.tile([65, STRIP], f32, tag="bank")
                    o_ps1 = bank.tile([65, STRIP], f32, tag="bank")
                    o_ps = [o_ps0, o_ps1]
                    for a in range(a_max + 1):
                        lo = max((a - 4 * s) * KT, 0) if causal else 0
                        sp = sps.tile([128, 2, STRIP], f32, tag="sps")
                        for i in range(2):
                            pr = slice(i * 64, (i + 1) * 64)
                            nc.tensor.matmul(
                                sp[:, i, lo:STRIP],
                                lhsT=k_s[a // 4][pr, hp, (a % 4) * KT:(a % 4 + 1) * KT],
                                rhs=q_s[s][pr, hp, lo:STRIP],
                                start=True, stop=True,
                            )
                        pt = ptile.tile([128, 2, STRIP], bf16, tag="pt")
                        nc.scalar.activation(out=pt[:, :, lo:STRIP],
                                             in_=sp[:, :, lo:STRIP],
                                             func=EXP, scale=0.125)
                        if causal and 0 <= a - 4 * s <= 3:
                            d0 = (a - 4 * s) * KT
                            for i in range(2):
                                nc.vector.tensor_mul(pt[:, i, d0:d0 + KT],
                                                     pt[:, i, d0:d0 + KT], tri_t)
                        if not causal:
                            mt = mtile.tile([128, STRIP], bf16, tag="mt")
                            nc.sync.dma_start(
                                out=mt, in_=maskT[a * KT:(a + 1) * KT, q0:q0 + STRIP])
                            for i in range(2):
                                nc.vector.tensor_mul(pt[:, i, :], pt[:, i, :], mt)
                        for i in range(2):
                            nc.tensor.matmul(o_ps[i][:, lo:STRIP],
                                             lhsT=v_s[a // 4][:, a % 4, 2 * hp + i, :],
                                             rhs=pt[:, i, lo:STRIP],
                                             start=(a == 0), stop=(a == a_max))
                    # normalize: recip of denom row, broadcast over 64 partitions
                    for i in range(2):
                        h = 2 * hp + i
                        r_t = small.tile([1, STRIP], f32r, tag="recip")
                        with nc.allow_low_precision(reason="float32r is fp32 bits"):
                            if causal:
                                nc.vector.reciprocal(r_t, o_ps[i][64:65, :])
                            else:
                                dn = small.tile([1, STRIP], f32r, tag="denom")
                                nc.vector.tensor_scalar_max(dn, o_ps[i][64:65, :], 1e-30)
                                nc.vector.reciprocal(r_t, dn)
                        bc_ps = bank.tile([64, STRIP], f32, tag="bank")
                        nc.tensor.matmul(bc_ps, lhsT=ones_r[0:1, 0:64], rhs=r_t,
                                         start=True, stop=True)
                        bc_t = small.tile([64, STRIP], f32, tag="bc")
                        nc.vector.tensor_copy(bc_t, bc_ps)
                        nc.vector.tensor_mul(
                            o_s[s][i * 64:(i + 1) * 64, hp, :],
                            o_ps[i][0:64, :], bc_t)

                ctx_.__exit__(None, None, None)

            def wo_strip(s):
                ctx_ = nc.named_scope(f"wo{s}"); ctx_.__enter__()
                st = stage.tile([128, 4, D], f16, tag="st")
                for t4 in range(4):
                    csl = slice(t4 * 128, (t4 + 1) * 128)
                    for n in range(2):
                        wps = bank.tile([128, 512], f32, tag="bank")
                        nsl = slice(n * 512, (n + 1) * 512)
                        for c in range(2):
                            nc.tensor.matmul(wps, lhsT=o_s[s][:, c, csl],
                                             rhs=wo_t[:, c, nsl],
                                             start=(c == 0), stop=(c == 1))
                        if (t4 + n) % 2 == 0:
                            nc.scalar.copy(out=st[:, t4, nsl], in_=wps)
                        else:
                            nc.vector.tensor_copy(st[:, t4, nsl], wps)
                nc.sync.dma_start(
                    out=out[s * STRIP:(s + 1) * STRIP, :].rearrange(
                        "(t p) n -> p t n", p=128),
                    in_=st,
                )
                ctx_.__exit__(None, None, None)

            if causal:
                proj_q(0)
                proj_kv(0)
                proj_q(1)
                proj_kv(1)
                with tc.high_priority():
                    attention_hp(0, 0)
                proj_q(2)
                with tc.high_priority():
                    attention_hp(0, 1)
                proj_k(2)
                wo_strip(0)
                with tc.high_priority():
                    attention_hp(1, 0)
                proj_v(2)
                proj_q(3)
                with tc.high_priority():
                    attention_hp(1, 1)
                proj_k(3)
                wo_strip(1)
                with tc.high_priority():
                    attention_hp(2, 0)
                proj_v(3)
                with tc.high_priority():
                    attention_hp(2, 1)
                wo_strip(2)
                with tc.high_priority():
                    attention_hp(3, 0)
                    attention_hp(3, 1)
                wo_strip(3)
            else:
                # non-causal: every strip reads every K/V tile, so all
                # projections must be emitted before any attention
                for t in range(NSUB):
                    proj_q(t)
                    proj_kv(t)
                for s in range(NSTRIP):
                    attention_hp(s, 0)
                    attention_hp(s, 1)
                    wo_strip(s)

    nc.compile()
    return nc


def _get_kernel(causal: bool):
    key = ("attn", causal)
    if key not in _CACHE:
        _CACHE[key] = _build(causal)
    return _CACHE[key]


def kernel(query, key, value, mask, wq, bq, wk, bk, wv, bv, wo, bo):
    import ml_dtypes
    from concourse import bass_utils

    f32 = np.float32
    bf16 = ml_dtypes.bfloat16

    mask_b = np.asarray(mask, dtype=bool)
    causal = bool(
        (mask_b[:, 0] == np.tril(np.ones((L, L), dtype=bool))[None]).all()
    )
    nc = _get_kernel(causal)

    tri_np = np.triu(np.ones((KT, KT), dtype=f32)).astype(bf16)
    qT = [np.ascontiguousarray(np.asarray(query[b], f32).T).astype(bf16) for b in range(B)]
    kT = [np.ascontiguousarray(np.asarray(key[b], f32).T).astype(bf16) for b in range(B)]
    vT = [np.ascontiguousarray(np.asarray(value[b], f32).T).astype(bf16) for b in range(B)]
    if not causal:
        maskT = [
            np.ascontiguousarray(mask_b[b, 0].T).astype(bf16) for b in range(B)
        ]

    wq = np.asarray(wq, f32)
    wk = np.asarray(wk, f32)
    wv = np.asarray(wv, f32)
    wo = np.asarray(wo, f32)
    bq = np.asarray(bq, f32)
    bk = np.asarray(bk, f32)
    bv = np.asarray(bv, f32)

    in_maps = []
    for c in range(NCORES):
        b, g = c // TP, c % TP
        gs = slice(g * DH, (g + 1) * DH)
        m = {
            "qT": qT[b], "kT": kT[b], "vT": vT[b],
            "wq": np.ascontiguousarray(wq[:, gs]).astype(bf16),
            "wk": np.ascontiguousarray(wk[:, gs]).astype(bf16),
            "wv": np.ascontiguousarray(wv[:, gs]).astype(bf16),
            "wo": np.ascontiguousarray(wo[gs, :]).astype(bf16),
            "bq": np.ascontiguousarray(bq[gs]),
            "bk": np.ascontiguousarray(bk[gs]),
            "bv": np.ascontiguousarray(bv[gs]).astype(bf16),
            "tri": tri_np,
        }
        if not causal:
            m["maskT"] = maskT[b]
        in_maps.append(m)

    res = bass_utils.run_bass_kernel_spmd(nc, in_maps, core_ids=list(range(NCORES)))

    out = np.zeros((B, L, D), f32)
    for c in range(NCORES):
        out[c // TP] += res.results[c]["out"].astype(f32)
    out += np.asarray(bo, f32)[None, None, :]
    return out


# revision 45
# speedup vs baseline: 1.0451x; 1.0140x over previous
"""Multi-head causal attention (B=2, L=2048, D=1024, H=16) on 8 trn2 cores.

Sharding: data-parallel over batch (2) x tensor-parallel over heads (4 groups
of 4 heads).  Core c handles batch c//4, heads 4*(c%4) .. 4*(c%4)+3.
Wq/Wk/Wv are column-sharded, Wo row-sharded; the TP all-reduce after Wo is
done host-side (sum of the 4 partial outputs per batch), as is the bo add.

Per-core kernel (matmul operands bf16, all accumulation fp32 in PSUM):
  - host supplies X^T (D on partitions) so projections need no on-chip
    transpose: Q^T/K^T = matmul(lhsT=W, rhs=X^T), V = matmul(lhsT=X^T, rhs=W)
  - S^T[k, q] tiles = matmul(lhsT=K^T tile, rhs=Q^T strip); softmax is
    computed WITHOUT max-subtraction (scores are ~N(0,0.6), bounded by ~4,
    so exp is safe) which keeps all reductions off the partition axis
  - P^T = exp(S^T/8) directly in the layout PV needs; a ones-row appended to
    V (lhsT [128, 65]) makes the PV matmul also emit softmax denominators
  - normalization: reciprocal of the denom row, partition-broadcast via a
    K=1 matmul (ones x recip), multiplied into O^T before the Wo matmul
  - causal mask: S^T/exp/PV restricted to q >= k-tile start; the single
    diagonal 128x128 block is masked by multiplying P^T with an upper-
    triangular 0/1 tile (supplied as input)
  - two heads are processed per S^T psum tile so each exp instruction covers
    [128, 2, 512-lo] (ACT per-instruction overhead is ~370ns)
"""

import numpy as np

B, L, D, H = 2, 2048, 1024, 16
DK = D // H          # 64
NCORES = 8
TP = 4               # head-group shards per batch
HG = H // TP         # 4 heads per core
DH = HG * DK         # 256 per-core head dims
STRIP = 512          # attention q-strip width
NSTRIP = L // STRIP  # 4
SUB = 512            # projection substrip width
NSUB = L // SUB      # 4
KT = 128             # key tile
NKT = L // KT        # 16

_CACHE = {}


def _build(causal: bool):
    import concourse.mybir as mybir
    import concourse.tile as tile
    from concourse import bacc

    f32 = mybir.dt.float32
    f32r = mybir.dt.float32r
    bf16 = mybir.dt.bfloat16
    EXP = mybir.ActivationFunctionType.Exp

    nc = bacc.Bacc("TRN2", target_bir_lowering=False)

    qT = nc.dram_tensor("qT", [D, L], bf16, kind="ExternalInput")
    kT = nc.dram_tensor("kT", [D, L], bf16, kind="ExternalInput")
    vT = nc.dram_tensor("vT", [D, L], bf16, kind="ExternalInput")
    wq = nc.dram_tensor("wq", [D, DH], bf16, kind="ExternalInput")
    wk = nc.dram_tensor("wk", [D, DH], bf16, kind="ExternalInput")
    wv = nc.dram_tensor("wv", [D, DH], bf16, kind="ExternalInput")
    wo = nc.dram_tensor("wo", [DH, D], bf16, kind="ExternalInput")
    bq = nc.dram_tensor("bq", [DH], f32, kind="ExternalInput")
    bk = nc.dram_tensor("bk", [DH], f32, kind="ExternalInput")
    bv = nc.dram_tensor("bv", [DH], bf16, kind="ExternalInput")
    tri = nc.dram_tensor("tri", [KT, KT], bf16, kind="ExternalInput")
    maskT = None
    if not causal:
        maskT = nc.dram_tensor("maskT", [L, L], bf16, kind="ExternalInput")
    f16 = mybir.dt.float16
    out = nc.dram_tensor("out", [L, D], f16, kind="ExternalOutput")

    with tile.TileContext(nc) as tc:
        with (
            tc.tile_pool(name="consts", bufs=1) as consts,
            tc.tile_pool(name="resident", bufs=1) as resident,
            tc.tile_pool(name="xin", bufs=2) as xin,
            tc.tile_pool(name="ptile", bufs=6) as ptile,
            tc.tile_pool(name="small", bufs=2) as small,
            tc.tile_pool(name="stage", bufs=2) as stage,
            tc.tile_pool(name="mtile", bufs=4) as mtile,
            tc.tile_pool(name="bank", bufs=4, space="PSUM") as bank,
            tc.tile_pool(name="sps", bufs=2, space="PSUM") as sps,
        ):
            # ---- constants / weights ----
            wq_t = consts.tile([128, 8, DH], bf16, tag="wq")
            wk_t = consts.tile([128, 8, DH], bf16, tag="wk")
            wv_t = consts.tile([128, 8, DH], bf16, tag="wv")
            wo_t = consts.tile([128, 2, D], bf16, tag="wo")
            xa = xin.tile([128, 8, SUB], bf16, tag="xq", name="xq_pre0")
            xb = xin.tile([128, 8, SUB], bf16, tag="xk", name="xk_pre0")
            xc = xin.tile([128, 8, SUB], bf16, tag="xv", name="xv_pre0")
            x_first = [(xa, xb, xc)]
            ssl = slice(0, SUB)
            nc.sync.dma_start(out=wq_t, in_=wq.rearrange("(c p) m -> p c m", p=128))
            nc.sync.dma_start(out=x_first[0][0], in_=qT[:, ssl].rearrange("(c p) n -> p c n", p=128))
            nc.sync.dma_start(out=wk_t, in_=wk.rearrange("(c p) m -> p c m", p=128))
            nc.sync.dma_start(out=x_first[0][1], in_=kT[:, ssl].rearrange("(c p) n -> p c n", p=128))
            nc.sync.dma_start(out=wv_t, in_=wv.rearrange("(c p) m -> p c m", p=128))
            nc.sync.dma_start(out=x_first[0][2], in_=vT[:, ssl].rearrange("(c p) n -> p c n", p=128))
            # per-partition bias columns for the q/k PSUM->SBUF copy
            bqP = consts.tile([128, 2], f32, tag="bqP")
            bkP = consts.tile([128, 2], f32, tag="bkP")
            nc.sync.dma_start(out=bqP, in_=bq.rearrange("(m p) -> p m", p=128))
            nc.sync.dma_start(out=bkP, in_=bk.rearrange("(m p) -> p m", p=128))
            bv_t = consts.tile([1, DH], bf16, tag="bv")
            nc.sync.dma_start(out=bv_t, in_=bv[:].unsqueeze(0))
            tri_t = consts.tile([KT, KT], bf16, tag="tri")
            nc.sync.dma_start(out=tri_t, in_=tri[:])
            nc.sync.dma_start(out=wo_t, in_=wo.rearrange("(c p) n -> p c n", p=128))
            ones_f = consts.tile([1, 128], f32, tag="ones")
            nc.vector.memset(ones_f, 1.0)
            ones_r = ones_f[:, :].bitcast(f32r)
            ones_b = consts.tile([1, 128], bf16, tag="onesb")
            nc.vector.memset(ones_b, 1.0)

            # ---- resident activations (one tile per strip to keep
            # scheduler dependencies fine-grained) ----
            # q_s/k_s: [partition = (h%2)*64 + dk, pair = h//2, q-in-strip]
            q_s, k_s, v_s, o_s = [], [], [], []
            for s in range(NSTRIP):
                q_tile = resident.tile([128, 2, STRIP], bf16, tag=f"q{s}", name=f"q{s}")
                k_tile = resident.tile([128, 2, STRIP], bf16, tag=f"k{s}", name=f"k{s}")
                # v: [partition = k within tile, ktile-in-strip, head, dk+1]
                v_tile = resident.tile([128, 4, HG, DK + 1], bf16, tag=f"v{s}", name=f"v{s}")
                nc.vector.memset(v_tile[:, :, :, DK : DK + 1], 1.0)
                o_tile = resident.tile([128, 2, STRIP], bf16, tag=f"o{s}", name=f"o{s}")
                q_s.append(q_tile); k_s.append(k_tile); v_s.append(v_tile); o_s.append(o_tile)

            def proj_q(t):
                ctx_ = nc.named_scope(f"projq{t}"); ctx_.__enter__()
                if t == 0:
                    x_q = x_first[0][0]
                else:
                    x_q = xin.tile([128, 8, SUB], bf16, tag="xq")
                    sl = slice(t * SUB, (t + 1) * SUB)
                    nc.sync.dma_start(out=x_q, in_=qT[:, sl].rearrange("(c p) n -> p c n", p=128))
                for m in range(2):
                    ps = bank.tile([128, SUB], f32, tag="bank")
                    msl = slice(m * 128, (m + 1) * 128)
                    for c in range(8):
                        nc.tensor.matmul(ps, lhsT=wq_t[:, c, msl], rhs=x_q[:, c, :],
                                         start=(c == 0), stop=(c == 7))
                    nc.vector.tensor_scalar_add(q_s[t][:, m, :], ps, bqP[:, m:m + 1])
                ctx_.__exit__(None, None, None)

            def proj_k(t):
                ctx_ = nc.named_scope(f"projk{t}"); ctx_.__enter__()
                if t == 0:
                    x_k = x_first[0][1]
                else:
                    x_k = xin.tile([128, 8, SUB], bf16, tag="xk")
                    sl = slice(t * SUB, (t + 1) * SUB)
                    nc.sync.dma_start(out=x_k, in_=kT[:, sl].rearrange("(c p) n -> p c n", p=128))
                for m in range(2):
                    ps = bank.tile([128, SUB], f32, tag="bank")
                    msl = slice(m * 128, (m + 1) * 128)
                    for c in range(8):
                        nc.tensor.matmul(ps, lhsT=wk_t[:, c, msl], rhs=x_k[:, c, :],
                                         start=(c == 0), stop=(c == 7))
                    nc.vector.tensor_scalar_add(k_s[t][:, m, :], ps, bkP[:, m:m + 1])
                ctx_.__exit__(None, None, None)

            def proj_v(t):
                ctx_ = nc.named_scope(f"projv{t}"); ctx_.__enter__()
                if t == 0:
                    x_v = x_first[0][2]
                else:
                    x_v = xin.tile([128, 8, SUB], bf16, tag="xv")
                    sl = slice(t * SUB, (t + 1) * SUB)
                    nc.sync.dma_start(out=x_v, in_=vT[:, sl].rearrange("(c p) n -> p c n", p=128))
                for j in range(4):
                    ps = bank.tile([128, DH], f32, tag="bank")
                    nc.tensor.matmul(ps, lhsT=ones_b, rhs=bv_t,
                                     start=True, stop=False)
                    for c in range(8):
                        nc.tensor.matmul(ps, lhsT=x_v[:, c, j * 128:(j + 1) * 128],
                                         rhs=wv_t[:, c, :], start=False, stop=(c == 7))
                    nc.vector.tensor_copy(
                        v_s[t][:, j, :, 0:DK],
                        ps.rearrange("p (h d) -> p h d", h=HG),
                    )
                ctx_.__exit__(None, None, None)

            def proj_kv(t):
                proj_k(t)
                proj_v(t)

            def attention_hp(s, hp):
                ctx_ = nc.named_scope(f"attn{s}h{hp}"); ctx_.__enter__()
                q0 = s * STRIP
                a_max = 4 * s + 3 if causal else NKT - 1
                if True:
                    o_ps0 = bank.tile([65, STRIP], f32, tag="bank")
                    o_ps1 = bank.tile([65, STRIP], f32, tag="bank")
                    o_ps = [o_ps0, o_ps1]
                    for a in range(a_max + 1):
                        lo = max((a - 4 * s) * KT, 0) if causal else 0
                        sp = sps.tile([128, 2, STRIP], f32, tag="sps")
                        for i in range(2):
                            pr = slice(i * 64, (i + 1) * 64)
                            nc.tensor.matmul(
                                sp[:, i, lo:STRIP],
                                lhsT=k_s[a // 4][pr, hp, (a % 4) * KT:(a % 4 + 1) * KT],
                                rhs=q_s[s][pr, hp, lo:STRIP],
                                start=True, stop=True,
                            )
                        pt = ptile.tile([128, 2, STRIP], bf16, tag="pt")
                        nc.scalar.activation(out=pt[:, :, lo:STRIP],
                                             in_=sp[:, :, lo:STRIP],
                                             func=EXP, scale=0.125)
                        if causal and 0 <= a - 4 * s <= 3:
                            d0 = (a - 4 * s) * KT
                            for i in range(2):
                                nc.vector.tensor_mul(pt[:, i, d0:d0 + KT],
                                                     pt[:, i, d0:d0 + KT], tri_t)
                        if not causal:
                            mt = mtile.tile([128, STRIP], bf16, tag="mt")
                            nc.sync.dma_start(
                                out=mt, in_=maskT[a * KT:(a + 1) * KT, q0:q0 + STRIP])
                            for i in range(2):
                                nc.vector.tensor_mul(pt[:, i, :], pt[:, i, :], mt)
                        for i in range(2):
                            nc.tensor.matmul(o_ps[i][:, lo:STRIP],
                                             lhsT=v_s[a // 4][:, a % 4, 2 * hp + i, :],
                                             rhs=pt[:, i, lo:STRIP],
                                             start=(a == 0), stop=(a == a_max))
                    # normalize: recip of denom row, broadcast over 64 partitions
                    for i in range(2):
                        h = 2 * hp + i
                        r_t = small.tile([1, STRIP], f32r, tag="recip")
                        with nc.allow_low_precision(reason="float32r is fp32 bits"):
                            if causal:
                                nc.vector.reciprocal(r_t, o_ps[i][64:65, :])
                            else:
                                dn = small.tile([1, STRIP], f32r, tag="denom")
                                nc.vector.tensor_scalar_max(dn, o_ps[i][64:65, :], 1e-30)
                                nc.vector.reciprocal(r_t, dn)
                        bc_ps = bank.tile([64, STRIP], f32, tag="bank")
                        nc.tensor.matmul(bc_ps, lhsT=ones_r[0:1, 0:64], rhs=r_t,
                                         start=True, stop=True)
                        bc_t = small.tile([64, STRIP], f32, tag="bc")
                        nc.vector.tensor_copy(bc_t, bc_ps)
                        nc.vector.tensor_mul(
                            o_s[s][i * 64:(i + 1) * 64, hp, :],
                            o_ps[i][0:64, :], bc_t)

                ctx_.__exit__(None, None, None)

            def wo_strip(s):
                ctx_ = nc.named_scope(f"wo{s}"); ctx_.__enter__()
                st = stage.tile([128, 4, D], f16, tag="st")
                for t4 in range(4):
                    csl = slice(t4 * 128, (t4 + 1) * 128)
                    for n in range(2):
                        wps = bank.tile([128, 512], f32, tag="bank")
                        nsl = slice(n * 512, (n + 1) * 512)
                        for c in range(2):
                            nc.tensor.matmul(wps, lhsT=o_s[s][:, c, csl],
                                             rhs=wo_t[:, c, nsl],
                                             start=(c == 0), stop=(c == 1))
                        if (t4 + n) % 2 == 0:
                            nc.scalar.copy(out=st[:, t4, nsl], in_=wps)
                        else:
                            nc.vector.tensor_copy(st[:, t4, nsl], wps)
                for half in range(2):
                    r0 = s * STRIP + half * 256
                    nc.sync.dma_start(
                        out=out[r0:r0 + 256, :].rearrange("(t p) n -> p t n", p=128),
                        in_=st[:, half * 2:(half + 1) * 2, :],
                    )
                ctx_.__exit__(None, None, None)

            if causal:
                proj_q(0)
                proj_kv(0)
                proj_q(1)
                proj_kv(1)
                with tc.high_priority():
                    attention_hp(0, 0)
                proj_q(2)
                with tc.high_priority():
                    attention_hp(0, 1)
                proj_k(2)
                wo_strip(0)
                with tc.high_priority():
                    attention_hp(1, 0)
                proj_v(2)
                proj_q(3)
                with tc.high_priority():
                    attention_hp(1, 1)
                proj_k(3)
                wo_strip(1)
                with tc.high_priority():
                    attention_hp(2, 0)
                proj_v(3)
                with tc.high_priority():
                    attention_hp(2, 1)
                wo_strip(2)
                with tc.high_priority():
                    attention_hp(3, 0)
                    attention_hp(3, 1)
                wo_strip(3)
            else:
                # non-causal: every strip reads every K/V tile, so all
                # projections must be emitted before any attention
                for t in range(NSUB):
                    proj_q(t)
                    proj_kv(t)
                for s in range(NSTRIP):
                    attention_hp(s, 0)
                    attention_hp(s, 1)
                    wo_strip(s)

    nc.compile()
    return nc


def _get_kernel(causal: bool):
    key = ("attn", causal)
    if key not in _CACHE:
        _CACHE[key] = _build(causal)
    return _CACHE[key]


def kernel(query, key, value, mask, wq, bq, wk, bk, wv, bv, wo, bo):
    import ml_dtypes
    from concourse import bass_utils

    f32 = np.float32
    bf16 = ml_dtypes.bfloat16

    mask_b = np.asarray(mask, dtype=bool)
    causal = bool(
        (mask_b[:, 0] == np.tril(np.ones((L, L), dtype=bool))[None]).all()
    )
    nc = _get_kernel(causal)

    tri_np = np.triu(np.ones((KT, KT), dtype=f32)).astype(bf16)
    qT = [np.ascontiguousarray(np.asarray(query[b], f32).T).astype(bf16) for b in range(B)]
    kT = [np.ascontiguousarray(np.asarray(key[b], f32).T).astype(bf16) for b in range(B)]
    vT = [np.ascontiguousarray(np.asarray(value[b], f32).T).astype(bf16) for b in range(B)]
    if not causal:
        maskT = [
            np.ascontiguousarray(mask_b[b, 0].T).astype(bf16) for b in range(B)
        ]

    wq = np.asarray(wq, f32)
    wk = np.asarray(wk, f32)
    wv = np.asarray(wv, f32)
    wo = np.asarray(wo, f32)
    bq = np.asarray(bq, f32)
    bk = np.asarray(bk, f32)
    bv = np.asarray(bv, f32)

    in_maps = []
    for c in range(NCORES):
        b, g = c // TP, c % TP
        gs = slice(g * DH, (g + 1) * DH)
        m = {
            "qT": qT[b], "kT": kT[b], "vT": vT[b],
            "wq": np.ascontiguousarray(wq[:, gs]).astype(bf16),
            "wk": np.ascontiguousarray(wk[:, gs]).astype(bf16),
            "wv": np.ascontiguousarray(wv[:, gs]).astype(bf16),
            "wo": np.ascontiguousarray(wo[gs, :]).astype(bf16),
            "bq": np.ascontiguousarray(bq[gs]),
            "bk": np.ascontiguousarray(bk[gs]),
            "bv": np.ascontiguousarray(bv[gs]).astype(bf16),
            "tri": tri_np,
        }
        if not causal:
            m["maskT"] = maskT[b]
        in_maps.append(m)

    res = bass_utils.run_bass_kernel_spmd(nc, in_maps, core_ids=list(range(NCORES)))

    out = np.zeros((B, L, D), f32)
    for c in range(NCORES):
        out[c // TP] += res.results[c]["out"].astype(f32)
    out += np.asarray(bo, f32)[None, None, :]
    return out
